# revision 17
# baseline (speedup 1.0000x reference)
"""Trainium2 Bass kernel for nn_BHLinear: x -> D0 -> FWHT/64 -> D1 -> FWHT/64 -> final_B.

Math (per row, f = 12-bit feature index = 64*u + v, u = 2m+j):
  FWHT_4096 = H64(u) (x) H64(v).  H64(v) folds into the adjacent block matmuls
  (C1 = H64@B1/64 per-u; G = H-half@final_B/64 per-out-block).  The remaining
  H64(u) = H2(j) (x) H32(m): H2 folds into the block-diag passes (P1/P3), and
  H32 runs as fixed-weight passes (P2/P4).

Five TensorE passes per tile [128 part, 16384 cols]; between passes the
contracted bits must move from columns onto partitions.  Two mechanisms, used
on different tiles to spread load across engine queues:

- "xbar" tiles: DMA xbar 128x128 block transposes (dma_start_transpose, all 16
  SDMA engines, ~10us transfer but ~19us of descriptor-gen on the issuing Sync
  queue).  Layout L_X(b): col = rh*128 + b*4 + rl (r = rh*4 + rl); a block
  transpose swaps w=(b,rl) with the 7 partition bits.  P2/P4 use kron(H32,I4).
  P1/P3 drain with a reorder (msub,rh,rl)->(rh,msub,rl) (measured free) and
  P3/P5 read 4-contig/128-stride rhs (measured full-rate).
- "dve" tiles: DVE 32x32 stream transposes of fp16 pairs viewed as int32
  (~14us each on the Vector queue).  T1/T3 write contiguous (rp,tl)-interleave;
  T2/T4 write the L2 interleave col = rph*512 + b*16 + rpl*2 + r01 so P3/P5
  read 16-contig runs (full rate).  P2/P4 use kron(I4,H32).

Drains (PSUM->SBUF, the other big cost: 1 elem/cycle/lane) alternate between
Scalar (~1.97us/2048) and Vector (~2.33us/2048) per an engine pattern tuned so
ACT ~ DVE.  Output fp16.  Sharding: rows split contiguously across 8 cores.
Host packs x into [T=4, 128, 32*R] fp16 (p=(j,v), col = m*512 + r) and unpacks
out [T, 128, 32*R] fp16 (p=(j'',o), col = W*512 + r, f = 64*(2W+j'')+o).
"""

import numpy as np

# ---- static config ---------------------------------------------------------
NCORES = 8
R = 256                  # rows per tile
T = 8                    # tiles per core
ROWS_PER_CORE = R * T    # 2048
D = 4096
TOTAL_ROWS = NCORES * ROWS_PER_CORE  # 16384

_F16 = np.float16


def _hadamard(n):
    H = np.array([[1.0]], dtype=np.float64)
    while H.shape[0] < n:
        H = np.block([[H, H], [H, -H]])
    return H


_H2 = _hadamard(2)
_H32 = _hadamard(32)
_H64 = _hadamard(64)


def _build_weights(inner_B, final_B):
    """w1/w3/w5 [128,4096] fp16 (32 lhsT blocks), w2x/w2d [128,128]."""
    B0 = inner_B[0].astype(np.float64)
    B1 = inner_B[1].astype(np.float64)
    fB = final_B.astype(np.float64)

    C1 = np.einsum('vk,ukt->uvt', _H64, B1) / 64.0
    G = np.zeros((64, 64, 64))
    for u in range(64):
        for h in range(2):
            G[u][:, 32 * h:32 * h + 32] = _H64[:, 32 * h:32 * h + 32] @ fB[2 * u + h] / 64.0

    w1 = np.zeros((128, 32, 128))
    w3 = np.zeros((128, 32, 128))
    w5 = np.zeros((128, 32, 128))
    for m in range(32):
        for j in range(2):
            for jp in range(2):
                w1[j * 64:(j + 1) * 64, m, jp * 64:(jp + 1) * 64] = _H2[j, jp] * B0[2 * m + j]
                w3[j * 64:(j + 1) * 64, m, jp * 64:(jp + 1) * 64] = _H2[j, jp] * C1[2 * m + j]
        for jpp in range(2):
            w5[jpp * 64:(jpp + 1) * 64, m, jpp * 64:(jpp + 1) * 64] = G[2 * m + jpp]
    w1 = w1.reshape(128, 4096)
    w3 = w3.reshape(128, 4096)
    w5 = w5.reshape(128, 4096)
    w2x = np.kron(_H32, np.eye(4))      # xbar tiles: partitions (m, rl)
    w2d = np.kron(np.eye(4), _H32)      # dve tiles: partitions (j', v'5, m)
    return (w1.astype(_F16), w2x.astype(_F16), w2d.astype(_F16),
            w3.astype(_F16), w5.astype(_F16))


def _pack_x(x):
    """x [..., 4096] fp32 -> list of per-core arrays [T, 128, 32*R] fp16.

    Partition p = (j, v); col = m*512 + r (layout A)."""
    xf = np.ascontiguousarray(x.reshape(-1, D))
    assert xf.shape[0] == TOTAL_ROWS
    x6 = xf.reshape(NCORES, T, R, 32, 2, 64)       # core,t,r,m,j,v
    x6 = x6.transpose(0, 1, 4, 5, 3, 2)            # core,t,j,v,m,r
    x6 = np.ascontiguousarray(x6).reshape(NCORES, T, 128, 32 * R)
    return [np.ascontiguousarray(x6[c]).astype(_F16) for c in range(NCORES)]


def _unpack_out(outs, orig_shape):
    """outs: list of per-core [T, 128, 32*R] fp16 -> [*orig_shape[:-1], 4096]."""
    o = np.stack(outs, axis=0).astype(np.float32)  # [core, T, 128, 32R]
    o = o.reshape(NCORES, T, 2, 64, 32, R)         # core,t,j'',o,W,r
    o = o.transpose(0, 1, 5, 4, 2, 3)              # core,t,r,W,j'',o
    o = np.ascontiguousarray(o).reshape(TOTAL_ROWS, D)
    return o.reshape(*orig_shape[:-1], D)


# ---- bass program ----------------------------------------------------------
_PROGRAM = None


def _build_program():
    global _PROGRAM
    if _PROGRAM is not None:
        return _PROGRAM
    from contextlib import ExitStack
    import concourse.tile as tile
    from concourse import bacc, mybir

    f32 = mybir.dt.float32
    f16 = mybir.dt.float16
    i32 = mybir.dt.int32

    nc = bacc.Bacc()
    x_d = nc.declare_dram_parameter("x", [T, 128, 32 * R], f16, isOutput=False)
    w1_d = nc.declare_dram_parameter("w1", [128, 4096], f16, isOutput=False)
    w2x_d = nc.declare_dram_parameter("w2x", [128, 128], f16, isOutput=False)
    w2d_d = nc.declare_dram_parameter("w2d", [128, 128], f16, isOutput=False)
    w3_d = nc.declare_dram_parameter("w3", [128, 4096], f16, isOutput=False)
    w5_d = nc.declare_dram_parameter("w5", [128, 4096], f16, isOutput=False)
    out_d = nc.declare_dram_parameter("out", [T, 128, 32 * R], f16, isOutput=True)

    C = 32 * R          # 8192 cols per tile
    QC = 8 * R          # x-quarter cols = 2048
    G = C // 2048       # psum groups per pass-tile = 4
    M = 32 // G         # m-blocks per psum group = 8

    with tile.TileContext(nc) as tc, ExitStack() as ctx:
        wpool = ctx.enter_context(tc.tile_pool(name="weights", bufs=1))
        xt_pool = ctx.enter_context(tc.tile_pool(name="xt", bufs=6))
        y_pool = ctx.enter_context(tc.tile_pool(name="y", bufs=9))
        out_pool = ctx.enter_context(tc.tile_pool(name="outp", bufs=2))
        psum = ctx.enter_context(tc.tile_pool(name="ps", bufs=2, space="PSUM"))
        yraw_pool = y_pool
        yt_pool = y_pool

        w1_sb = wpool.tile([128, 4096], f16)
        w2x_sb = wpool.tile([128, 128], f16)
        w2d_sb = wpool.tile([128, 128], f16)
        w3_sb = wpool.tile([128, 4096], f16)
        w5_sb = wpool.tile([128, 4096], f16)
        nc.sync.dma_start(w1_sb[:], w1_d[:])
        nc.sync.dma_start(w2x_sb[:], w2x_d[:])
        nc.sync.dma_start(w2d_sb[:], w2d_d[:])
        nc.sync.dma_start(w3_sb[:], w3_d[:])
        nc.sync.dma_start(w5_sb[:], w5_d[:])

        sc = nc.scalar.copy
        vc = nc.vector.tensor_copy

        def pick(eng, g):
            return eng if eng in (sc, vc) else eng[g % len(eng)]

        # -------- xbar-chain helpers --------
        def xbar_T(dst, src):
            nc.sync.dma_start_transpose(
                dst[:].rearrange("p (g c) -> p g c", c=128), src[:])

        def emit_perm_x(src_fn, w_sb, dst, eng):
            """P1/P3 xbar-style: contiguous psum slabs; reordering drain
            (msub, rh, rl) -> (rh, msub, rl) produces L_X."""
            for g in range(G):
                ps = psum.tile([128, 2048], f32, tag="ps")
                for i in range(M):
                    b = M * g + i
                    nc.tensor.matmul(
                        ps[:, i * R:(i + 1) * R],
                        w_sb[:, b * 128:(b + 1) * 128],
                        src_fn(b),
                        start=True, stop=True,
                    )
                dst_v = dst[:].rearrange("p (rh mq rl) -> p rh mq rl",
                                         mq=32, rl=4)[:, :, M * g:M * (g + 1), :]
                src_v = ps[:].rearrange("p (msub rh rl) -> p rh msub rl",
                                        msub=M, rl=4)
                pick(eng, g)(dst_v, src_v)

        def emit_fixed(src, w_sb, dst, eng):
            """P2/P4 (either chain): contiguous rhs chunks and drains."""
            for g in range(G):
                ps = psum.tile([128, 2048], f32, tag="ps")
                for i in range(4):
                    nc.tensor.matmul(
                        ps[:, i * 512:(i + 1) * 512],
                        w_sb[:],
                        src[:, g * 2048 + i * 512:g * 2048 + (i + 1) * 512],
                        start=True, stop=True,
                    )
                pick(eng, g)(dst[:, g * 2048:(g + 1) * 2048], ps[:])

        def rhs4(src):
            return src[:].rearrange("p (rh m rl) -> p m rh rl", m=32, rl=4)  # rh=C//128

        def emit_p5_x(y4t, t, eng):
            y4v = rhs4(y4t)
            for g in range(G):
                out_sb = out_pool.tile([128, 2048], f16, tag="outp")
                ps = psum.tile([128, 2048], f32, tag="ps")
                for i in range(M):
                    W = M * g + i
                    nc.tensor.matmul(
                        ps[:, i * R:(i + 1) * R],
                        w5_sb[:, W * 128:(W + 1) * 128],
                        y4v[:, W],
                        start=True, stop=True,
                    )
                pick(eng, g)(out_sb[:], ps[:])
                nc.scalar.dma_start(out_d[t][:, g * 2048:(g + 1) * 2048], out_sb[:])

        # -------- dve-chain helpers --------
        def pair_T_cc(dst, src):
            """A-layout in (m-blocked) -> contiguous (rp, tl) interleave out."""
            in_v = src[:].bitcast(i32).rearrange("p (m rp) -> p rp m", m=32)
            out_v = dst[:].bitcast(i32).rearrange("p (rp tl) -> p rp tl", tl=32)
            nc.vector.transpose(out_v, in_v)

        def pair_T_l2(dst, src):
            """(rp, tl)-interleave in -> L2 out: ci = rph*256 + b*8 + rpl."""
            in_v = src[:].bitcast(i32).rearrange("p (rp tl) -> p rp tl", tl=32)
            out_v = dst[:].bitcast(i32).rearrange(
                "p (rph tl rpl) -> p rph rpl tl", rph=C // 512, tl=32, rpl=8)
            nc.vector.transpose(out_v, in_v)

        def rhs16(src):
            # L2 fp16: c = rph*512 + b*16 + q (q = rpl*2 + r01, 16-contig)
            return src[:].rearrange("p (rph m q) -> p m rph q", rph=C // 512, m=32)

        def emit_perm_d(src_fn, w_sb, dst, eng):
            """P1/P3 dve-style: contiguous psum slabs and drains -> layout A."""
            for g in range(G):
                ps = psum.tile([128, 2048], f32, tag="ps")
                for i in range(M):
                    b = M * g + i
                    nc.tensor.matmul(
                        ps[:, i * R:(i + 1) * R],
                        w_sb[:, b * 128:(b + 1) * 128],
                        src_fn(b),
                        start=True, stop=True,
                    )
                pick(eng, g)(dst[:, g * 2048:(g + 1) * 2048], ps[:])

        def emit_p5_d(y4t, t, eng):
            y4v = rhs16(y4t)
            for g in range(G):
                out_sb = out_pool.tile([128, 2048], f16, tag="outp")
                ps = psum.tile([128, 2048], f32, tag="ps")
                for i in range(M):
                    W = M * g + i
                    nc.tensor.matmul(
                        ps[:, i * R:(i + 1) * R],
                        w5_sb[:, W * 128:(W + 1) * 128],
                        y4v[:, W],
                        start=True, stop=True,
                    )
                pick(eng, g)(out_sb[:], ps[:])
                nc.scalar.dma_start(out_d[t][:, g * 2048:(g + 1) * 2048], out_sb[:])

        def load_x(t):
            xq = []
            for q in range(4):
                xt = xt_pool.tile([128, QC], f16, tag="xt")
                nc.sync.dma_start(xt[:], x_d[t][:, q * QC:(q + 1) * QC])
                xq.append(xt)
            return xq

        # engine patterns: sc-heavy on dve tiles (DVE also does transposes)
        X_PAT = (sc, vc, sc, vc, sc, vc, sc, vc)       # 4:4 on xbar tiles
        D_PAT = (sc, sc, vc, sc, sc, vc, sc, sc)       # 6:2 on dve tiles

        def run_xbar_tile(y, xq, t):
            y['1'] = yraw_pool.tile([128, C], f16, tag="y", name="y1")
            emit_perm_x(lambda b: xq[b // 8][:, (b % 8) * R:(b % 8 + 1) * R],
                        w1_sb, y['1'], X_PAT)
            y['1t'] = yt_pool.tile([128, C], f16, tag="y", name="y1t")
            xbar_T(y['1t'], y['1'])
            y['2'] = yraw_pool.tile([128, C], f16, tag="y", name="y2")
            emit_fixed(y['1t'], w2x_sb, y['2'], X_PAT)
            y['2t'] = yt_pool.tile([128, C], f16, tag="y", name="y2t")
            xbar_T(y['2t'], y['2'])
            y['3'] = yraw_pool.tile([128, C], f16, tag="y", name="y3")
            emit_perm_x(lambda b, s=y['2t']: rhs4(s)[:, b], w3_sb, y['3'], X_PAT)
            y['3t'] = yt_pool.tile([128, C], f16, tag="y", name="y3t")
            xbar_T(y['3t'], y['3'])
            y['4'] = yraw_pool.tile([128, C], f16, tag="y", name="y4")
            emit_fixed(y['3t'], w2x_sb, y['4'], X_PAT)
            y['4t'] = yt_pool.tile([128, C], f16, tag="y", name="y4t")
            xbar_T(y['4t'], y['4'])
            emit_p5_x(y['4t'], t, X_PAT)

        def run_dve_tile(y, xq, t):
            y['1'] = yraw_pool.tile([128, C], f16, tag="y", name="y1")
            emit_perm_d(lambda b: xq[b // 8][:, (b % 8) * R:(b % 8 + 1) * R],
                        w1_sb, y['1'], D_PAT)
            y['1t'] = yt_pool.tile([128, C], f16, tag="y", name="y1t")
            pair_T_cc(y['1t'], y['1'])
            y['2'] = yraw_pool.tile([128, C], f16, tag="y", name="y2")
            emit_fixed(y['1t'], w2d_sb, y['2'], D_PAT)
            y['2t'] = yt_pool.tile([128, C], f16, tag="y", name="y2t")
            pair_T_l2(y['2t'], y['2'])
            y['3'] = yraw_pool.tile([128, C], f16, tag="y", name="y3")
            emit_perm_d(lambda b, s=y['2t']: rhs16(s)[:, b], w3_sb, y['3'], D_PAT)
            y['3t'] = yt_pool.tile([128, C], f16, tag="y", name="y3t")
            pair_T_cc(y['3t'], y['3'])
            y['4'] = yraw_pool.tile([128, C], f16, tag="y", name="y4")
            emit_fixed(y['3t'], w2d_sb, y['4'], D_PAT)
            y['4t'] = yt_pool.tile([128, C], f16, tag="y", name="y4t")
            pair_T_l2(y['4t'], y['4'])
            emit_p5_d(y['4t'], t, D_PAT)

        # interleave one xbar tile with one dve tile per pair, stage by stage
        STAGES = ('s1', 'st1', 's2', 'st2', 's3', 'st3', 's4', 'st4', 's5')

        def stage(kind, y, xq, t, s):
            if s == 's1':
                y['1'] = yraw_pool.tile([128, C], f16, tag="y", name="y1")
                emit = emit_perm_x if kind == 'x' else emit_perm_d
                emit(lambda b: xq[b // 8][:, (b % 8) * R:(b % 8 + 1) * R],
                     w1_sb, y['1'], X_PAT if kind == 'x' else D_PAT)
            elif s == 'st1':
                y['1t'] = yt_pool.tile([128, C], f16, tag="y", name="y1t")
                (xbar_T if kind == 'x' else pair_T_cc)(y['1t'], y['1'])
            elif s == 's2':
                y['2'] = yraw_pool.tile([128, C], f16, tag="y", name="y2")
                emit_fixed(y['1t'], w2x_sb if kind == 'x' else w2d_sb, y['2'],
                           X_PAT if kind == 'x' else D_PAT)
            elif s == 'st2':
                y['2t'] = yt_pool.tile([128, C], f16, tag="y", name="y2t")
                (xbar_T if kind == 'x' else pair_T_l2)(y['2t'], y['2'])
            elif s == 's3':
                y['3'] = yraw_pool.tile([128, C], f16, tag="y", name="y3")
                if kind == 'x':
                    emit_perm_x(lambda b, s_=y['2t']: rhs4(s_)[:, b], w3_sb,
                                y['3'], X_PAT)
                else:
                    emit_perm_d(lambda b, s_=y['2t']: rhs16(s_)[:, b], w3_sb,
                                y['3'], D_PAT)
            elif s == 'st3':
                y['3t'] = yt_pool.tile([128, C], f16, tag="y", name="y3t")
                (xbar_T if kind == 'x' else pair_T_cc)(y['3t'], y['3'])
            elif s == 's4':
                y['4'] = yraw_pool.tile([128, C], f16, tag="y", name="y4")
                emit_fixed(y['3t'], w2x_sb if kind == 'x' else w2d_sb, y['4'],
                           X_PAT if kind == 'x' else D_PAT)
            elif s == 'st4':
                y['4t'] = yt_pool.tile([128, C], f16, tag="y", name="y4t")
                (xbar_T if kind == 'x' else pair_T_l2)(y['4t'], y['4'])
            elif s == 's5':
                (emit_p5_x if kind == 'x' else emit_p5_d)(
                    y['4t'], t, X_PAT if kind == 'x' else D_PAT)

        for tp in range(T // 2):
            ta, tb = 2 * tp, 2 * tp + 1          # ta: xbar chain, tb: dve chain
            xqa = load_x(ta)
            xqb = load_x(tb)
            ya = {}
            yb = {}
            for s in STAGES:
                stage('x', ya, xqa, ta, s)
                stage('d', yb, xqb, tb, s)

    nc.finalize()
    _PROGRAM = nc
    return nc


_LAST_RESULTS = None


def _make_in_maps(x, inner_B, final_B):
    w1, w2x, w2d, w3, w5 = _build_weights(np.asarray(inner_B), np.asarray(final_B))
    x_packed = _pack_x(np.asarray(x, dtype=np.float32))
    return [
        {"x": x_packed[c], "w1": w1, "w2x": w2x, "w2d": w2d, "w3": w3, "w5": w5}
        for c in range(NCORES)
    ]


def kernel(x, inner_B, final_B, _trace=False):
    global _LAST_RESULTS
    from concourse.bass_utils import run_bass_kernel_spmd

    orig_shape = x.shape
    in_maps = _make_in_maps(x, inner_B, final_B)

    nc = _build_program()
    try:
        res = run_bass_kernel_spmd(nc, in_maps, list(range(NCORES)))
    except Exception:
        # transient NRT device errors have been observed; retry once
        res = run_bass_kernel_spmd(nc, in_maps, list(range(NCORES)))
    _LAST_RESULTS = res
    outs = [np.asarray(res.results[c]["out"]) for c in range(NCORES)]
    return _unpack_out(outs, orig_shape).astype(np.float32)


# revision 18
# speedup vs baseline: 1.2473x; 1.2473x over previous
"""Trainium2 Bass kernel for nn_BHLinear: x -> D0 -> FWHT/64 -> D1 -> FWHT/64 -> final_B.

Math (per row, f = 12-bit feature index = 64*u + v, u = 2m+j):
  FWHT_4096 = H64(u) (x) H64(v).  H64(v) folds into the adjacent block matmuls
  (C1 = H64@B1/64 per-u; G = H-half@final_B/64 per-out-block).  The remaining
  H64(u) = H2(j) (x) H32(m): H2 folds into the block-diag passes (P1/P3), and
  H32 runs as fixed-weight passes (P2/P4).

Five TensorE passes per tile [128 part, 16384 cols]; between passes the
contracted bits must move from columns onto partitions.  Two mechanisms, used
on different tiles to spread load across engine queues:

- "xbar" tiles: DMA xbar 128x128 block transposes (dma_start_transpose, all 16
  SDMA engines, ~10us transfer but ~19us of descriptor-gen on the issuing Sync
  queue).  Layout L_X(b): col = rh*128 + b*4 + rl (r = rh*4 + rl); a block
  transpose swaps w=(b,rl) with the 7 partition bits.  P2/P4 use kron(H32,I4).
  P1/P3 drain with a reorder (msub,rh,rl)->(rh,msub,rl) (measured free) and
  P3/P5 read 4-contig/128-stride rhs (measured full-rate).
- "dve" tiles: DVE 32x32 stream transposes of fp16 pairs viewed as int32
  (~14us each on the Vector queue).  T1/T3 write contiguous (rp,tl)-interleave;
  T2/T4 write the L2 interleave col = rph*512 + b*16 + rpl*2 + r01 so P3/P5
  read 16-contig runs (full rate).  P2/P4 use kron(I4,H32).

Drains (PSUM->SBUF, the other big cost: 1 elem/cycle/lane) alternate between
Scalar (~1.97us/2048) and Vector (~2.33us/2048) per an engine pattern tuned so
ACT ~ DVE.  Output fp16.  Sharding: rows split contiguously across 8 cores.
Host packs x into [T=4, 128, 32*R] fp16 (p=(j,v), col = m*512 + r) and unpacks
out [T, 128, 32*R] fp16 (p=(j'',o), col = W*512 + r, f = 64*(2W+j'')+o).
"""

import numpy as np

# ---- static config ---------------------------------------------------------
NCORES = 8
R = 512                  # rows per tile
T = 4                    # tiles per core
ROWS_PER_CORE = R * T    # 2048
D = 4096
TOTAL_ROWS = NCORES * ROWS_PER_CORE  # 16384

_F16 = np.float16


def _hadamard(n):
    H = np.array([[1.0]], dtype=np.float64)
    while H.shape[0] < n:
        H = np.block([[H, H], [H, -H]])
    return H


_H2 = _hadamard(2)
_H32 = _hadamard(32)
_H64 = _hadamard(64)


def _build_weights(inner_B, final_B):
    """w1/w3/w5 [128,4096] fp16 (32 lhsT blocks), w2x/w2d [128,128]."""
    B0 = inner_B[0].astype(np.float64)
    B1 = inner_B[1].astype(np.float64)
    fB = final_B.astype(np.float64)

    C1 = np.einsum('vk,ukt->uvt', _H64, B1) / 64.0
    G = np.zeros((64, 64, 64))
    for u in range(64):
        for h in range(2):
            G[u][:, 32 * h:32 * h + 32] = _H64[:, 32 * h:32 * h + 32] @ fB[2 * u + h] / 64.0

    w1 = np.zeros((128, 32, 128))
    w3 = np.zeros((128, 32, 128))
    w5 = np.zeros((128, 32, 128))
    for m in range(32):
        for j in range(2):
            for jp in range(2):
                w1[j * 64:(j + 1) * 64, m, jp * 64:(jp + 1) * 64] = _H2[j, jp] * B0[2 * m + j]
                w3[j * 64:(j + 1) * 64, m, jp * 64:(jp + 1) * 64] = _H2[j, jp] * C1[2 * m + j]
        for jpp in range(2):
            w5[jpp * 64:(jpp + 1) * 64, m, jpp * 64:(jpp + 1) * 64] = G[2 * m + jpp]
    w1 = w1.reshape(128, 4096)
    w3 = w3.reshape(128, 4096)
    w5 = w5.reshape(128, 4096)
    w2x = np.kron(_H32, np.eye(4))      # xbar tiles: partitions (m, rl)
    w2d = np.kron(np.eye(4), _H32)      # dve tiles: partitions (j', v'5, m)
    return (w1.astype(_F16), w2x.astype(_F16), w2d.astype(_F16),
            w3.astype(_F16), w5.astype(_F16))


def _pack_x(x):
    """x [..., 4096] fp32 -> list of per-core arrays [T, 128, 32*R] fp16.

    Partition p = (j, v); col = m*512 + r (layout A)."""
    xf = np.ascontiguousarray(x.reshape(-1, D))
    assert xf.shape[0] == TOTAL_ROWS
    x6 = xf.reshape(NCORES, T, R, 32, 2, 64)       # core,t,r,m,j,v
    x6 = x6.transpose(0, 1, 4, 5, 3, 2)            # core,t,j,v,m,r
    x6 = np.ascontiguousarray(x6).reshape(NCORES, T, 128, 32 * R)
    return [np.ascontiguousarray(x6[c]).astype(_F16) for c in range(NCORES)]


def _unpack_out(outs, orig_shape):
    """outs: list of per-core [T, 128, 32*R] fp16 -> [*orig_shape[:-1], 4096]."""
    o = np.stack(outs, axis=0).astype(np.float32)  # [core, T, 128, 32R]
    o = o.reshape(NCORES, T, 2, 64, 32, R)         # core,t,j'',o,W,r
    o = o.transpose(0, 1, 5, 4, 2, 3)              # core,t,r,W,j'',o
    o = np.ascontiguousarray(o).reshape(TOTAL_ROWS, D)
    return o.reshape(*orig_shape[:-1], D)


# ---- bass program ----------------------------------------------------------
_PROGRAM = None


def _build_program():
    global _PROGRAM
    if _PROGRAM is not None:
        return _PROGRAM
    from contextlib import ExitStack
    import concourse.tile as tile
    from concourse import bacc, mybir

    f32 = mybir.dt.float32
    f16 = mybir.dt.float16
    i32 = mybir.dt.int32

    nc = bacc.Bacc()
    x_d = nc.declare_dram_parameter("x", [T, 128, 32 * R], f16, isOutput=False)
    w1_d = nc.declare_dram_parameter("w1", [128, 4096], f16, isOutput=False)
    w2x_d = nc.declare_dram_parameter("w2x", [128, 128], f16, isOutput=False)
    w2d_d = nc.declare_dram_parameter("w2d", [128, 128], f16, isOutput=False)
    w3_d = nc.declare_dram_parameter("w3", [128, 4096], f16, isOutput=False)
    w5_d = nc.declare_dram_parameter("w5", [128, 4096], f16, isOutput=False)
    out_d = nc.declare_dram_parameter("out", [T, 128, 32 * R], f16, isOutput=True)

    C = 32 * R          # 16384 cols per tile
    QC = 8 * R          # x-quarter cols = 4096

    with tile.TileContext(nc) as tc, ExitStack() as ctx:
        wpool = ctx.enter_context(tc.tile_pool(name="weights", bufs=1))
        xt_pool = ctx.enter_context(tc.tile_pool(name="xt", bufs=5))
        yraw_pool = ctx.enter_context(tc.tile_pool(name="yraw", bufs=2))
        yt_pool = ctx.enter_context(tc.tile_pool(name="yt", bufs=2))
        out_pool = ctx.enter_context(tc.tile_pool(name="outp", bufs=2))
        psum = ctx.enter_context(tc.tile_pool(name="ps", bufs=2, space="PSUM"))

        w1_sb = wpool.tile([128, 4096], f16)
        w2x_sb = wpool.tile([128, 128], f16)
        w2d_sb = wpool.tile([128, 128], f16)
        w3_sb = wpool.tile([128, 4096], f16)
        w5_sb = wpool.tile([128, 4096], f16)
        nc.sync.dma_start(w1_sb[:], w1_d[:])
        nc.sync.dma_start(w2x_sb[:], w2x_d[:])
        nc.sync.dma_start(w2d_sb[:], w2d_d[:])
        nc.sync.dma_start(w3_sb[:], w3_d[:])
        nc.sync.dma_start(w5_sb[:], w5_d[:])

        sc = nc.scalar.copy
        vc = nc.vector.tensor_copy

        def pick(eng, g):
            return eng if eng in (sc, vc) else eng[g % len(eng)]

        # -------- xbar-chain helpers --------
        def xbar_T(dst, src):
            H = C // 2
            for h in range(2):
                nc.sync.dma_start_transpose(
                    dst[:, h * H:(h + 1) * H].rearrange("p (g c) -> p g c", c=128),
                    src[:, h * H:(h + 1) * H])

        def emit_perm_x(src_fn, w_sb, dst, eng):
            """P1/P3 xbar-style: contiguous psum slabs; reordering drain
            (msub, rh, rl) -> (rh, msub, rl) produces L_X."""
            for g in range(8):
                ps = psum.tile([128, 2048], f32, tag="ps")
                for i in range(4):
                    b = 4 * g + i
                    nc.tensor.matmul(
                        ps[:, i * R:(i + 1) * R],
                        w_sb[:, b * 128:(b + 1) * 128],
                        src_fn(b),
                        start=True, stop=True,
                    )
                dst_v = dst[:].rearrange("p (rh mq rl) -> p rh mq rl",
                                         mq=32, rl=4)[:, :, 4 * g:4 * (g + 1), :]
                src_v = ps[:].rearrange("p (msub rh rl) -> p rh msub rl",
                                        msub=4, rl=4)
                pick(eng, g)(dst_v, src_v)

        def emit_fixed(src, w_sb, dst, eng):
            """P2/P4 (either chain): contiguous rhs chunks and drains."""
            for g in range(8):
                ps = psum.tile([128, 2048], f32, tag="ps")
                for i in range(4):
                    b = 4 * g + i
                    nc.tensor.matmul(
                        ps[:, i * R:(i + 1) * R],
                        w_sb[:],
                        src[:, b * R:(b + 1) * R],
                        start=True, stop=True,
                    )
                pick(eng, g)(dst[:, g * 2048:(g + 1) * 2048], ps[:])

        def rhs4(src):
            return src[:].rearrange("p (rh m rl) -> p m rh rl", m=32, rl=4)

        def emit_p5_x(y4t, t, eng):
            y4v = rhs4(y4t)
            for g in range(8):
                out_sb = out_pool.tile([128, 2048], f16, tag="outp")
                ps = psum.tile([128, 2048], f32, tag="ps")
                for i in range(4):
                    W = 4 * g + i
                    nc.tensor.matmul(
                        ps[:, i * R:(i + 1) * R],
                        w5_sb[:, W * 128:(W + 1) * 128],
                        y4v[:, W],
                        start=True, stop=True,
                    )
                pick(eng, g)(out_sb[:], ps[:])
                nc.sync.dma_start(out_d[t][:, g * 2048:(g + 1) * 2048], out_sb[:])

        # -------- dve-chain helpers --------
        def pair_T_cc(dst, src):
            """A-layout in (m-blocked) -> contiguous (rp, tl) interleave out."""
            in_v = src[:].bitcast(i32).rearrange("p (m rp) -> p rp m", m=32)
            out_v = dst[:].bitcast(i32).rearrange("p (rp tl) -> p rp tl", tl=32)
            hp = 128  # rp half
            for h in range(2):
                nc.vector.transpose(out_v[:, h * hp:(h + 1) * hp, :],
                                    in_v[:, h * hp:(h + 1) * hp, :])

        def pair_T_l2(dst, src):
            """(rp, tl)-interleave in -> L2 out: ci = rph*256 + b*8 + rpl."""
            in_v = src[:].bitcast(i32).rearrange("p (rp tl) -> p rp tl", tl=32)
            out_v = dst[:].bitcast(i32).rearrange(
                "p (rph tl rpl) -> p rph rpl tl", rph=32, tl=32, rpl=8)
            for h in range(2):
                nc.vector.transpose(out_v[:, h * 16:(h + 1) * 16, :, :],
                                    in_v[:, h * 128:(h + 1) * 128, :])

        def rhs16(src):
            # L2 fp16: c = rph*512 + b*16 + q (q = rpl*2 + r01, 16-contig)
            return src[:].rearrange("p (rph m q) -> p m rph q", rph=32, m=32)

        def emit_perm_d(src_fn, w_sb, dst, eng):
            """P1/P3 dve-style: contiguous psum slabs and drains -> layout A."""
            for g in range(8):
                ps = psum.tile([128, 2048], f32, tag="ps")
                for i in range(4):
                    b = 4 * g + i
                    nc.tensor.matmul(
                        ps[:, i * R:(i + 1) * R],
                        w_sb[:, b * 128:(b + 1) * 128],
                        src_fn(b),
                        start=True, stop=True,
                    )
                pick(eng, g)(dst[:, g * 2048:(g + 1) * 2048], ps[:])

        def emit_p5_d(y4t, t, eng):
            y4v = rhs16(y4t)
            for g in range(8):
                out_sb = out_pool.tile([128, 2048], f16, tag="outp")
                ps = psum.tile([128, 2048], f32, tag="ps")
                for i in range(4):
                    W = 4 * g + i
                    nc.tensor.matmul(
                        ps[:, i * R:(i + 1) * R],
                        w5_sb[:, W * 128:(W + 1) * 128],
                        y4v[:, W],
                        start=True, stop=True,
                    )
                pick(eng, g)(out_sb[:], ps[:])
                nc.sync.dma_start(out_d[t][:, g * 2048:(g + 1) * 2048], out_sb[:])

        def load_x(t):
            xq = []
            for q in range(4):
                xt = xt_pool.tile([128, QC], f16, tag="xt")
                nc.sync.dma_start(xt[:], x_d[t][:, q * QC:(q + 1) * QC])
                xq.append(xt)
            return xq

        # engine patterns: sc-heavy on dve tiles (DVE also does transposes)
        X_PAT = (sc, vc, sc, sc, vc, sc, sc, vc)       # 5:3 on xbar tiles
        D_PAT = (sc, sc, vc, sc, sc, vc, sc, sc)       # 6:2 on dve tiles

        def run_xbar_tile(y, xq, t):
            y['1'] = yraw_pool.tile([128, C], f16, tag="yraw", name="y1")
            emit_perm_x(lambda b: xq[b // 8][:, (b % 8) * R:(b % 8 + 1) * R],
                        w1_sb, y['1'], X_PAT)
            y['1t'] = yt_pool.tile([128, C], f16, tag="yt", name="y1t")
            xbar_T(y['1t'], y['1'])
            y['2'] = yraw_pool.tile([128, C], f16, tag="yraw", name="y2")
            emit_fixed(y['1t'], w2x_sb, y['2'], X_PAT)
            y['2t'] = yt_pool.tile([128, C], f16, tag="yt", name="y2t")
            xbar_T(y['2t'], y['2'])
            y['3'] = yraw_pool.tile([128, C], f16, tag="yraw", name="y3")
            emit_perm_x(lambda b, s=y['2t']: rhs4(s)[:, b], w3_sb, y['3'], X_PAT)
            y['3t'] = yt_pool.tile([128, C], f16, tag="yt", name="y3t")
            xbar_T(y['3t'], y['3'])
            y['4'] = yraw_pool.tile([128, C], f16, tag="yraw", name="y4")
            emit_fixed(y['3t'], w2x_sb, y['4'], X_PAT)
            y['4t'] = yt_pool.tile([128, C], f16, tag="yt", name="y4t")
            xbar_T(y['4t'], y['4'])
            emit_p5_x(y['4t'], t, X_PAT)

        def run_dve_tile(y, xq, t):
            y['1'] = yraw_pool.tile([128, C], f16, tag="yraw", name="y1")
            emit_perm_d(lambda b: xq[b // 8][:, (b % 8) * R:(b % 8 + 1) * R],
                        w1_sb, y['1'], D_PAT)
            y['1t'] = yt_pool.tile([128, C], f16, tag="yt", name="y1t")
            pair_T_cc(y['1t'], y['1'])
            y['2'] = yraw_pool.tile([128, C], f16, tag="yraw", name="y2")
            emit_fixed(y['1t'], w2d_sb, y['2'], D_PAT)
            y['2t'] = yt_pool.tile([128, C], f16, tag="yt", name="y2t")
            pair_T_l2(y['2t'], y['2'])
            y['3'] = yraw_pool.tile([128, C], f16, tag="yraw", name="y3")
            emit_perm_d(lambda b, s=y['2t']: rhs16(s)[:, b], w3_sb, y['3'], D_PAT)
            y['3t'] = yt_pool.tile([128, C], f16, tag="yt", name="y3t")
            pair_T_cc(y['3t'], y['3'])
            y['4'] = yraw_pool.tile([128, C], f16, tag="yraw", name="y4")
            emit_fixed(y['3t'], w2d_sb, y['4'], D_PAT)
            y['4t'] = yt_pool.tile([128, C], f16, tag="yt", name="y4t")
            pair_T_l2(y['4t'], y['4'])
            emit_p5_d(y['4t'], t, D_PAT)

        # interleave one xbar tile with one dve tile per pair, stage by stage
        STAGES = ('s1', 'st1', 's2', 'st2', 's3', 'st3', 's4', 'st4', 's5')

        def stage(kind, y, xq, t, s):
            if s == 's1':
                y['1'] = yraw_pool.tile([128, C], f16, tag="yraw", name="y1")
                emit = emit_perm_x if kind == 'x' else emit_perm_d
                emit(lambda b: xq[b // 8][:, (b % 8) * R:(b % 8 + 1) * R],
                     w1_sb, y['1'], X_PAT if kind == 'x' else D_PAT)
            elif s == 'st1':
                y['1t'] = yt_pool.tile([128, C], f16, tag="yt", name="y1t")
                (xbar_T if kind == 'x' else pair_T_cc)(y['1t'], y['1'])
            elif s == 's2':
                y['2'] = yraw_pool.tile([128, C], f16, tag="yraw", name="y2")
                emit_fixed(y['1t'], w2x_sb if kind == 'x' else w2d_sb, y['2'],
                           X_PAT if kind == 'x' else D_PAT)
            elif s == 'st2':
                y['2t'] = yt_pool.tile([128, C], f16, tag="yt", name="y2t")
                (xbar_T if kind == 'x' else pair_T_l2)(y['2t'], y['2'])
            elif s == 's3':
                y['3'] = yraw_pool.tile([128, C], f16, tag="yraw", name="y3")
                if kind == 'x':
                    emit_perm_x(lambda b, s_=y['2t']: rhs4(s_)[:, b], w3_sb,
                                y['3'], X_PAT)
                else:
                    emit_perm_d(lambda b, s_=y['2t']: rhs16(s_)[:, b], w3_sb,
                                y['3'], D_PAT)
            elif s == 'st3':
                y['3t'] = yt_pool.tile([128, C], f16, tag="yt", name="y3t")
                (xbar_T if kind == 'x' else pair_T_cc)(y['3t'], y['3'])
            elif s == 's4':
                y['4'] = yraw_pool.tile([128, C], f16, tag="yraw", name="y4")
                emit_fixed(y['3t'], w2x_sb if kind == 'x' else w2d_sb, y['4'],
                           X_PAT if kind == 'x' else D_PAT)
            elif s == 'st4':
                y['4t'] = yt_pool.tile([128, C], f16, tag="yt", name="y4t")
                (xbar_T if kind == 'x' else pair_T_l2)(y['4t'], y['4'])
            elif s == 's5':
                (emit_p5_x if kind == 'x' else emit_p5_d)(
                    y['4t'], t, X_PAT if kind == 'x' else D_PAT)

        for tp in range(T // 2):
            ta, tb = 2 * tp, 2 * tp + 1          # ta: xbar chain, tb: dve chain
            xqa = load_x(ta)
            xqb = load_x(tb)
            ya = {}
            yb = {}
            for s in STAGES:
                stage('x', ya, xqa, ta, s)
                stage('d', yb, xqb, tb, s)

    nc.finalize()
    _PROGRAM = nc
    return nc


_LAST_RESULTS = None


def _make_in_maps(x, inner_B, final_B):
    w1, w2x, w2d, w3, w5 = _build_weights(np.asarray(inner_B), np.asarray(final_B))
    x_packed = _pack_x(np.asarray(x, dtype=np.float32))
    return [
        {"x": x_packed[c], "w1": w1, "w2x": w2x, "w2d": w2d, "w3": w3, "w5": w5}
        for c in range(NCORES)
    ]


def kernel(x, inner_B, final_B, _trace=False):
    global _LAST_RESULTS
    from concourse.bass_utils import run_bass_kernel_spmd

    orig_shape = x.shape
    in_maps = _make_in_maps(x, inner_B, final_B)

    nc = _build_program()
    try:
        res = run_bass_kernel_spmd(nc, in_maps, list(range(NCORES)))
    except Exception:
        # transient NRT device errors have been observed; retry once
        res = run_bass_kernel_spmd(nc, in_maps, list(range(NCORES)))
    _LAST_RESULTS = res
    outs = [np.asarray(res.results[c]["out"]) for c in range(NCORES)]
    return _unpack_out(outs, orig_shape).astype(np.float32)
